# revision 36
# baseline (speedup 1.0000x reference)
"""Trainium2 Bass kernel for nn_AttnDecoderCNN (3-step CNN attention decoder).

Contract: kernel(**inputs) takes the FULL unsharded inputs (as produced by
reference.setup_inputs()) and returns the FULL output tuple
(softmax(h3)[B,H,1], log_softmax(logits)[B,1,V]).

Strategy: pure data parallel over the batch dim across 8 NeuronCores; each
core handles BL=16 batch rows. Per layer (3 serial layers):
  - The K=3 conv reduces to one [e,b]x[e,o] matmul plus a precomputed
    constant (window columns 0/1 are the PAD embedding + positional encoding
    for every batch row and every layer) -> y_const, folded on the host.
  - scores[b,s] = sum_h d[b,h]*eo[b,h,s] on the PE. Compute engines can only
    address PSUM base partitions 0/32/64, so each b's contribution is an
    M=16 matmul whose stationary d-tile is masked to column b (cross-term
    rows become zeros) and ALL b accumulate into one [16,S] psum tile.
    eo is streamed as an (hi,lo) bf16 pair and d is split likewise; the
    three cross-term bf16 matmuls (hi*hi + hi*lo + lo*hi) give ~16-bit
    effective mantissa at 1/4 the fp32 PE cost. (fp32 matmul = 4 passes;
    fp32r is ~tf32 precision -> 1e-2 end-to-end error; both rejected.)
  - softmax along s: DVE reduce (negated max) + one ACT exp with accum_out,
    normalize in place.
  - context c[b,h] = sum_s attn[b,s]*et[b,h,s]: attn row b is broadcast to
    all 128 partitions with a one-hot selector matmul into a [128,S] psum
    tile, then one affine_mul_reduce per (b,h-chunk) multiplies the fp32
    et tile by the broadcast and accumulates the row sum directly into
    cT[hc][:,b]. (tensor_tensor_reduce hangs on this HW/compiler stack;
    affine_mul_reduce is the custom-DVE equivalent.)
  - h = glu + c feeds the next layer's window column. Everything stays in a
    transposed [h,b] layout so the serial path needs no transposes.
Encoder streams: eo(hi+lo bf16) on the sync-engine HWDGE ring, et(fp32) on
the scalar-engine ring, so both rings stream concurrently. The memory
roofline is 3 layers x 128MB per core.
"""

import numpy as np
import ml_dtypes
from contextlib import ExitStack

import concourse.bass as bass
import concourse.bacc as bacc
import concourse.mybir as mybir
from concourse.tile import TileContext
from concourse.bass_utils import run_bass_kernel_spmd

B, E, H, S, V, K = 128, 512, 512, 2048, 83, 3
PAD = 82
NCORES = 8
BL = B // NCORES          # 16 batch rows per core
F32 = mybir.dt.float32
BF16 = mybir.dt.bfloat16
HC = H // 128             # 4 h-chunks (partition tiles)
EC = E // 128             # 4 e-chunks
ST = S // 512             # 4 s-tiles of 512 (matmul free-dim limit)
OC = 2 * H // 128         # 8 conv-output chunks
AF = mybir.ActivationFunctionType
OP = mybir.AluOpType
AX = mybir.AxisListType

# knobs (full kernel: STAGE=99, NLAYERS=3, REPS=1)
STAGE = 99
NLAYERS = 3
REPS = 1


def _pos_emb(n, e):
    pos = np.arange(n, dtype=np.float32)[:, None]
    k = np.exp(-np.log(10000.0) * np.arange(0, e, 2, dtype=np.float32) / e)
    arg = pos * k
    pe = np.stack([np.sin(arg), np.cos(arg)], axis=-1).reshape(n, e)
    return pe.T.astype(np.float32)  # [e, n]


def _dump(nc, scr, out1, out2, tile_ap):
    z2 = scr.tile([BL, V], F32, name="z2", tag="z2")
    nc.vector.memset(z2[:], 0.0)
    p, f = min(tile_ap.shape[0], BL), min(tile_ap.shape[1], H)
    nc.sync.dma_start(out1[0:p, 0:f], tile_ap[0:p, 0:f])
    nc.sync.dma_start(out2[:], z2[:])


def _build_nc():
    nc = bacc.Bacc()
    eoh = nc.declare_dram_parameter("eoh", [BL, H, S], BF16, isOutput=False)
    eol = nc.declare_dram_parameter("eol", [BL, H, S], BF16, isOutput=False)
    et = nc.declare_dram_parameter("et", [BL, H, S], F32, isOutput=False)
    w2t = nc.declare_dram_parameter("w2t", [128, EC * 2 * H], F32, isOutput=False)
    l2t = nc.declare_dram_parameter("l2t", [128, HC * V], F32, isOutput=False)
    ycst = nc.declare_dram_parameter("ycst", [128, OC], F32, isOutput=False)
    pec2 = nc.declare_dram_parameter("pec2", [128, HC], F32, isOutput=False)
    t1t = nc.declare_dram_parameter("t1t", [128, HC * BL], F32, isOutput=False)
    ident = nc.declare_dram_parameter("ident", [128, 128], F32, isOutput=False)
    onehot = nc.declare_dram_parameter("onehot", [BL, BL * 128], F32, isOutput=False)
    colmask = nc.declare_dram_parameter("colmask", [128, BL * BL], BF16,
                                        isOutput=False)
    l2b = nc.declare_dram_parameter("l2b", [1, V], F32, isOutput=False)
    out1 = nc.declare_dram_parameter("out1", [BL, H], F32, isOutput=True)
    out2 = nc.declare_dram_parameter("out2", [BL, V], F32, isOutput=True)

    with ExitStack() as ctx:
        tc = ctx.enter_context(TileContext(nc))
        const = ctx.enter_context(tc.tile_pool(name="const", bufs=1))
        state = ctx.enter_context(tc.tile_pool(name="state", bufs=1))
        scr = ctx.enter_context(tc.tile_pool(name="scr", bufs=2))
        big = ctx.enter_context(tc.tile_pool(name="big", bufs=1))
        eop = ctx.enter_context(tc.tile_pool(name="eop", bufs=8))
        etp = ctx.enter_context(tc.tile_pool(name="etp", bufs=8))
        psp = ctx.enter_context(tc.tile_pool(name="psp", bufs=2, space="PSUM"))

        w2t_sb = const.tile([128, EC * 2 * H], F32, name="w2t_sb")
        nc.sync.dma_start(w2t_sb[:], w2t[:])
        l2t_sb = const.tile([128, HC * V], F32, name="l2t_sb")
        nc.sync.dma_start(l2t_sb[:], l2t[:])
        ycst_sb = const.tile([128, OC], F32, name="ycst_sb")
        nc.sync.dma_start(ycst_sb[:], ycst[:])
        pec2_sb = const.tile([128, HC], F32, name="pec2_sb")
        nc.sync.dma_start(pec2_sb[:], pec2[:])
        t1t_sb = const.tile([128, HC * BL], F32, name="t1t_sb")
        nc.sync.dma_start(t1t_sb[:], t1t[:])
        ident_sb = const.tile([128, 128], F32, name="ident_sb")
        nc.sync.dma_start(ident_sb[:], ident[:])
        onehot_sb = const.tile([BL, BL * 128], F32, name="onehot_sb")
        nc.sync.dma_start(onehot_sb[:], onehot[:])
        colmask_sb = const.tile([128, BL * BL], BF16, name="colmask_sb")
        nc.sync.dma_start(colmask_sb[:], colmask[:])
        l2b_sb = const.tile([1, V], F32, name="l2b_sb")
        nc.sync.dma_start(l2b_sb[:], l2b[:])
        ones_sb = const.tile([1, 128], F32, name="ones_sb")
        nc.vector.memset(ones_sb[:], 1.0)

        for rep in range(REPS):
            r = f"r{rep}_"
            tT = [t1t_sb[:, hc * BL:(hc + 1) * BL] for hc in range(HC)]
            hT = None
            for l in range(NLAYERS):
                # ---- conv (single live window column) + GLU + residual ----
                y_ps = psp.tile([128, OC * BL], F32, name=f"{r}y_ps{l}", tag="ps")
                for oc in range(OC):
                    for ec in range(EC):
                        nc.tensor.matmul(
                            y_ps[:, oc * BL:(oc + 1) * BL],
                            w2t_sb[:, ec * 2 * H + oc * 128:
                                   ec * 2 * H + (oc + 1) * 128],
                            tT[ec],
                            start=(ec == 0), stop=(ec == EC - 1))
                glu, dT = [], []
                for j in range(HC):
                    sig = scr.tile([128, BL], F32, name="sig", tag="sig")
                    nc.scalar.activation(sig[:],
                                         y_ps[:, (HC + j) * BL:(HC + j + 1) * BL],
                                         AF.Sigmoid,
                                         bias=ycst_sb[:, HC + j:HC + j + 1])
                    av = scr.tile([128, BL], F32, name="av", tag="av")
                    nc.scalar.activation(av[:], y_ps[:, j * BL:(j + 1) * BL],
                                         AF.Identity, bias=ycst_sb[:, j:j + 1])
                    g = state.tile([128, BL], F32, name=f"{r}glu{l}_{j}",
                                   tag=f"{r}glu{l}_{j}")
                    nc.vector.tensor_tensor(g[:], av[:], sig[:], op=OP.mult)
                    d = state.tile([128, BL], F32, name=f"{r}dT{l}_{j}",
                                   tag=f"{r}dT{l}_{j}")
                    nc.vector.tensor_tensor(d[:], g[:], tT[j], op=OP.add)
                    glu.append(g)
                    dT.append(d)

                # hi/lo bf16 split of d (~16-bit effective mantissa)
                dh, dl = [], []
                for j in range(HC):
                    h_ = state.tile([128, BL], BF16, name=f"{r}dh{l}_{j}",
                                    tag=f"{r}dh{l}_{j}")
                    nc.scalar.copy(h_[:], dT[j][:])
                    lo = state.tile([128, BL], BF16, name=f"{r}dl{l}_{j}",
                                    tag=f"{r}dl{l}_{j}")
                    nc.vector.tensor_tensor(lo[:], dT[j][:], h_[:], op=OP.subtract)
                    dh.append(h_)
                    dl.append(lo)

                if STAGE == 0:
                    _dump(nc, scr, out1, out2, dT[0])
                    return nc

                # ---- attention scores (three bf16 cross-term matmuls) ----
                sc_ps = psp.tile([BL, S], F32, name=f"{r}sc_ps{l}", tag="ps")
                for b in range(BL):
                    eoh_ts, eol_ts = [], []
                    eng = nc.sync
                    for hc in range(HC):
                        eh_t = eop.tile([128, S], BF16, name="eh_t", tag="eoh")
                        eng.dma_start(eh_t[:],
                                      eoh[b, hc * 128:(hc + 1) * 128, :])
                        el_t = eop.tile([128, S], BF16, name="el_t", tag="eol")
                        eng.dma_start(el_t[:],
                                      eol[b, hc * 128:(hc + 1) * 128, :])
                        eoh_ts.append(eh_t)
                        eol_ts.append(el_t)
                    dmh, dml = [], []
                    for hc in range(HC):
                        mh = scr.tile([128, BL], BF16, name="dmh",
                                      tag=f"dmh_{hc}", bufs=2)
                        nc.vector.tensor_tensor(
                            mh[:], dh[hc][:], colmask_sb[:, b * BL:(b + 1) * BL],
                            op=OP.mult)
                        ml_ = scr.tile([128, BL], BF16, name="dml",
                                       tag=f"dml_{hc}", bufs=2)
                        nc.vector.tensor_tensor(
                            ml_[:], dl[hc][:], colmask_sb[:, b * BL:(b + 1) * BL],
                            op=OP.mult)
                        dmh.append(mh)
                        dml.append(ml_)
                    for hc in range(HC):
                        for wt, rhs_t in ((dmh[hc], eoh_ts[hc]),
                                          (dmh[hc], eol_ts[hc]),
                                          (dml[hc], eoh_ts[hc])):
                            for st in range(ST):
                                sl = slice(st * 512, (st + 1) * 512)
                                start = (b == 0 and hc == 0
                                         and wt is dmh[hc] and rhs_t is eoh_ts[hc])
                                stop = (b == BL - 1 and hc == HC - 1
                                        and wt is dml[hc])
                                nc.tensor.matmul(sc_ps[:, sl], wt[:],
                                                 rhs_t[:, sl],
                                                 start=start, stop=stop)

                # ---- softmax along s (read directly from PSUM) ----
                negmax = scr.tile([BL, 1], F32, name="negmax", tag="negmax")
                nc.vector.tensor_reduce(negmax[:], sc_ps[:], axis=AX.X,
                                        op=OP.max, negate=True)
                expt = big.tile([BL, S], F32, name="expt", tag="expt")
                sumexp = scr.tile([BL, 1], F32, name="sumexp", tag="sumexp")
                nc.scalar.activation(expt[:], sc_ps[:], AF.Exp, bias=negmax[:],
                                     accum_out=sumexp[:])
                rsum = scr.tile([BL, 1], F32, name="rsum", tag="rsum")
                nc.vector.reciprocal(rsum[:], sumexp[:])
                attn = expt
                nc.vector.tensor_scalar_mul(attn[:], expt[:], rsum[:])

                if STAGE == 1:
                    _dump(nc, scr, out1, out2, attn)
                    return nc

                # ---- context: c[b,h] = sum_s attn[b,s] * et[b,h,s] ----
                cT = [state.tile([128, BL], F32, name=f"{r}cT{l}_{hc}",
                                 tag=f"{r}cT{l}_{hc}") for hc in range(HC)]
                for b in range(BL):
                    et_ts = []
                    eng = nc.scalar
                    for hc in range(HC):
                        et_t = etp.tile([128, S], F32, name="et_t", tag="et")
                        eng.dma_start(et_t[:],
                                      et[b, hc * 128:(hc + 1) * 128, :])
                        et_ts.append(et_t)
                    # broadcast attn row b to all 128 partitions (one-hot mm)
                    bc_ps = psp.tile([128, S], F32, name="bc_ps", tag="ps")
                    for st in range(ST):
                        sl = slice(st * 512, (st + 1) * 512)
                        nc.tensor.matmul(bc_ps[:, sl],
                                         onehot_sb[:, b * 128:(b + 1) * 128],
                                         attn[:, sl], start=True, stop=True)
                    for hc in range(HC):
                        ttr_out = scr.tile([128, S], F32, name="ttr_out",
                                           tag="ttr_out", bufs=1)
                        nc.vector.affine_mul_reduce(
                            out=ttr_out[:],
                            accum_out=cT[hc][:, b:b + 1],
                            in0=et_ts[hc][:],
                            in1=bc_ps[:],
                            scale=1.0, bias=0.0)

                if STAGE == 2:
                    _dump(nc, scr, out1, out2, cT[0])
                    return nc

                # ---- h = glu + c ; next-layer t = h + pe3[:,2] ----
                hT = []
                for hc in range(HC):
                    h = state.tile([128, BL], F32, name=f"{r}hT{l}_{hc}",
                                   tag=f"{r}hT{l}_{hc}")
                    nc.vector.tensor_tensor(h[:], glu[hc][:], cT[hc][:],
                                            op=OP.add)
                    hT.append(h)
                if l < NLAYERS - 1:
                    tT = []
                    for hc in range(HC):
                        t = state.tile([128, BL], F32, name=f"{r}tT{l + 1}_{hc}",
                                       tag=f"{r}tT{l + 1}_{hc}")
                        nc.vector.tensor_scalar_add(t[:], hT[hc][:],
                                                    pec2_sb[:, hc:hc + 1])
                        tT.append(t)

            # ---- out1 = softmax(h3, axis=h) ----
            eh = []
            for hc in range(HC):
                e_t = state.tile([128, BL], F32, name=f"{r}eh_{hc}",
                                 tag=f"{r}eh_{hc}")
                nc.scalar.activation(e_t[:], hT[hc][:], AF.Exp)
                eh.append(e_t)
            tp_ps = psp.tile([BL, H], F32, name=f"{r}tp_ps", tag="ps")
            for hc in range(HC):
                nc.tensor.matmul(tp_ps[:, hc * 128:(hc + 1) * 128], eh[hc][:],
                                 ident_sb[:], is_transpose=True,
                                 start=True, stop=True)
            se = scr.tile([BL, 1], F32, name="se", tag="se")
            nc.vector.reduce_sum(se[:], tp_ps[:], axis=AX.X)
            rse = scr.tile([BL, 1], F32, name="rse", tag="rse")
            nc.vector.reciprocal(rse[:], se[:])
            out1_sb = scr.tile([BL, H], F32, name="out1_sb", tag="out1_sb")
            nc.vector.tensor_scalar_mul(out1_sb[:], tp_ps[:], rse[:])
            nc.sync.dma_start(out1[:], out1_sb[:])

            # ---- out2 = log_softmax(h3 @ lin2_w.T + lin2_b) ----
            lg_ps = psp.tile([BL, V], F32, name=f"{r}lg_ps", tag="ps")
            for hc in range(HC):
                nc.tensor.matmul(lg_ps[:], hT[hc][:],
                                 l2t_sb[:, hc * V:(hc + 1) * V],
                                 start=(hc == 0), stop=False)
            nc.tensor.matmul(lg_ps[:], ones_sb[:, :BL], l2b_sb[:],
                             start=False, stop=True)
            negml = scr.tile([BL, 1], F32, name="negml", tag="negml")
            nc.vector.tensor_reduce(negml[:], lg_ps[:], axis=AX.X, op=OP.max,
                                    negate=True)
            exl = scr.tile([BL, V], F32, name="exl", tag="exl")
            sel = scr.tile([BL, 1], F32, name="sel", tag="sel")
            nc.scalar.activation(exl[:], lg_ps[:], AF.Exp, bias=negml[:],
                                 accum_out=sel[:])
            lsel = scr.tile([BL, 1], F32, name="lsel", tag="lsel")
            nc.scalar.activation(lsel[:], sel[:], AF.Ln)
            out2_sb = scr.tile([BL, V], F32, name="out2_sb", tag="out2_sb")
            nc.vector.tensor_scalar(out2_sb[:], lg_ps[:], negml[:], lsel[:],
                                    op0=OP.add, op1=OP.subtract)
            nc.sync.dma_start(out2[:], out2_sb[:])

    return nc


def _host_prep(inputs):
    pe3 = _pos_emb(K, E)                                    # [512, 3]
    emb = np.asarray(inputs["emb"], np.float32)
    conv_w = np.asarray(inputs["conv_w"], np.float32)
    conv_b = np.asarray(inputs["conv_b"], np.float32)
    l2w = np.asarray(inputs["lin2_w"], np.float32)
    l2bv = np.asarray(inputs["lin2_b"], np.float32)
    tok = np.asarray(inputs["decoder_input"]).astype(np.int64)

    c0 = emb[PAD] + pe3[:, 0]
    c1 = emb[PAD] + pe3[:, 1]
    y_const = conv_w[:, :, 0] @ c0 + conv_w[:, :, 1] @ c1 + conv_b   # [1024]
    w2 = conv_w[:, :, 2]                                    # [1024, 512]
    w2t = np.ascontiguousarray(
        np.transpose(w2.reshape(2 * H, EC, 128), (2, 1, 0)).reshape(128, EC * 2 * H))
    l2t = np.ascontiguousarray(
        np.transpose(l2w.reshape(V, HC, 128), (2, 1, 0)).reshape(128, HC * V))
    ycst = np.ascontiguousarray(y_const.reshape(OC, 128).T)
    pec2 = np.ascontiguousarray(pe3[:, 2].reshape(HC, 128).T)
    t1 = emb[tok] + pe3[:, 2][None, :]                      # [B, 512]
    ident = np.eye(128, dtype=np.float32)
    onehot = np.zeros((BL, BL * 128), dtype=np.float32)
    colmask = np.zeros((128, BL * BL), dtype=ml_dtypes.bfloat16)
    for b in range(BL):
        onehot[b, b * 128:(b + 1) * 128] = 1.0
        colmask[:, b * BL + b] = 1.0
    l2b = np.ascontiguousarray(l2bv.reshape(1, V))
    return w2t, l2t, ycst, pec2, t1, ident, onehot, colmask, l2b


def _make_in_maps(inputs):
    eo_full = np.asarray(inputs["encoder_output"], np.float32)
    eoh_full = eo_full.astype(ml_dtypes.bfloat16)
    eol_full = (eo_full - eoh_full.astype(np.float32)).astype(ml_dtypes.bfloat16)
    et_full = np.ascontiguousarray(np.asarray(inputs["encoder_total"], np.float32))
    w2t, l2t, ycst, pec2, t1, ident, onehot, colmask, l2b = _host_prep(inputs)
    in_maps = []
    for c in range(NCORES):
        sl = slice(c * BL, (c + 1) * BL)
        t1c = t1[sl]                                        # [16, 512]
        t1t = np.ascontiguousarray(
            np.transpose(t1c.reshape(BL, HC, 128), (2, 1, 0)).reshape(128, HC * BL))
        in_maps.append({
            "eoh": np.ascontiguousarray(eoh_full[sl]),
            "eol": np.ascontiguousarray(eol_full[sl]),
            "et": np.ascontiguousarray(et_full[sl]),
            "w2t": w2t, "l2t": l2t, "ycst": ycst, "pec2": pec2,
            "t1t": t1t, "ident": ident, "onehot": onehot, "colmask": colmask,
            "l2b": l2b,
        })
    return in_maps


_NC = None
LAST_RESULTS = None


def kernel(**inputs):
    global _NC, LAST_RESULTS
    import os
    in_maps = _make_in_maps(inputs)

    if _NC is None:
        _NC = _build_nc()
        _NC.finalize()   # Bacc.compile(): wait splitting, event sems, regalloc

    trace = bool(int(os.environ.get("KERNEL_TRACE", "0")))
    res = run_bass_kernel_spmd(_NC, in_maps, list(range(NCORES)), trace=trace)
    LAST_RESULTS = res
    out1 = np.concatenate([np.asarray(r["out1"]) for r in res.results], axis=0)
    out2 = np.concatenate([np.asarray(r["out2"]) for r in res.results], axis=0)
    return (np.ascontiguousarray(out1[:, :, None].astype(np.float32)),
            np.ascontiguousarray(out2[:, None, :].astype(np.float32)))


# revision 38
# speedup vs baseline: 1.2042x; 1.2042x over previous
"""Trainium2 Bass kernel for nn_AttnDecoderCNN (3-step CNN attention decoder).

Contract: kernel(**inputs) takes the FULL unsharded inputs (as produced by
reference.setup_inputs()) and returns the FULL output tuple
(softmax(h3)[B,H,1], log_softmax(logits)[B,1,V]).

Strategy: pure data parallel over the batch dim across 8 NeuronCores; each
core handles BL=16 batch rows. Per layer (3 serial layers):
  - The K=3 conv reduces to one [e,b]x[e,o] matmul plus a precomputed
    constant (window columns 0/1 are the PAD embedding + positional encoding
    for every batch row and every layer) -> y_const, folded on the host.
  - scores[b,s] = sum_h d[b,h]*eo[b,h,s] on the PE. Compute engines can only
    address PSUM base partitions 0/32/64, so each b's contribution is an
    M=16 matmul whose stationary d-tile is masked to column b (cross-term
    rows become zeros) and ALL b accumulate into one [16,S] psum tile.
    eo is streamed as an (hi,lo) bf16 pair and d is split likewise; the
    three cross-term bf16 matmuls (hi*hi + hi*lo + lo*hi) give ~16-bit
    effective mantissa at 1/4 the fp32 PE cost. (fp32 matmul = 4 passes;
    fp32r is ~tf32 precision -> 1e-2 end-to-end error; both rejected.)
  - softmax along s: DVE reduce (negated max) + one ACT exp with accum_out,
    normalize in place.
  - context c[b,h] = sum_s attn[b,s]*et[b,h,s]: attn row b is broadcast to
    all 128 partitions with a one-hot selector matmul into a [128,S] psum
    tile, then one affine_mul_reduce per (b,h-chunk) multiplies the fp32
    et tile by the broadcast and accumulates the row sum directly into
    cT[hc][:,b]. (tensor_tensor_reduce hangs on this HW/compiler stack;
    affine_mul_reduce is the custom-DVE equivalent.)
  - h = glu + c feeds the next layer's window column. Everything stays in a
    transposed [h,b] layout so the serial path needs no transposes.
Encoder streams: eo(hi+lo bf16) on the sync-engine HWDGE ring, et(fp32) on
the scalar-engine ring, so both rings stream concurrently. The memory
roofline is 3 layers x 128MB per core.
"""

import numpy as np
import ml_dtypes
from contextlib import ExitStack

import concourse.bass as bass
import concourse.bacc as bacc
import concourse.mybir as mybir
from concourse.tile import TileContext
from concourse.bass_utils import run_bass_kernel_spmd

B, E, H, S, V, K = 128, 512, 512, 2048, 83, 3
PAD = 82
NCORES = 8
BL = B // NCORES          # 16 batch rows per core
F32 = mybir.dt.float32
BF16 = mybir.dt.bfloat16
HC = H // 128             # 4 h-chunks (partition tiles)
EC = E // 128             # 4 e-chunks
ST = S // 512             # 4 s-tiles of 512 (matmul free-dim limit)
OC = 2 * H // 128         # 8 conv-output chunks
AF = mybir.ActivationFunctionType
OP = mybir.AluOpType
AX = mybir.AxisListType

# knobs (full kernel: STAGE=99, NLAYERS=3, REPS=1)
STAGE = 99
NLAYERS = 3
REPS = 1


def _patch_hwdge_lane_partition():
    """Partition the 8 DMAHW semaphore lanes by issuing engine (SP -> 0-3,
    Activation -> 4-7) instead of one global round-robin.

    Tile's stock assignment shares every lane between both HWDGE rings.
    Each lane's wait values assume FIFO completion in tick order, which
    only holds per-ring — two rings feeding one lane can satisfy a waiter
    before its actual producer finished (observed as a runtime crash when
    both rings stream within the same phase). Keying lanes on the engine
    restores per-lane single-ring FIFO and makes concurrent dual-ring
    streaming sound."""
    import concourse.tile_sem_assignment as tsa

    cls = tsa.TileClockTick
    if getattr(cls, "_lane_partition_patched", False):
        return
    orig = cls._assign_tick

    def patched(self, inst):
        eng = getattr(inst, "engine", None)
        if (isinstance(inst, tsa.DMAInst)
                and not isinstance(inst, tsa.bass_isa.UserSyncedRemoteDMADescs)
                and eng is not None
                and eng != tsa.mybir.EngineType.Pool):
            if eng == tsa.mybir.EngineType.SP:
                lane = getattr(self, "_sp_lane", 0)
                self._sp_lane = (lane + 1) % 4
                self.next_hw_dma_idx = lane
            else:
                lane = getattr(self, "_act_lane", 0)
                self._act_lane = (lane + 1) % 4
                self.next_hw_dma_idx = 4 + lane
        return orig(self, inst)

    cls._assign_tick = patched
    cls._lane_partition_patched = True


_patch_hwdge_lane_partition()


def _pos_emb(n, e):
    pos = np.arange(n, dtype=np.float32)[:, None]
    k = np.exp(-np.log(10000.0) * np.arange(0, e, 2, dtype=np.float32) / e)
    arg = pos * k
    pe = np.stack([np.sin(arg), np.cos(arg)], axis=-1).reshape(n, e)
    return pe.T.astype(np.float32)  # [e, n]


def _dump(nc, scr, out1, out2, tile_ap):
    z2 = scr.tile([BL, V], F32, name="z2", tag="z2")
    nc.vector.memset(z2[:], 0.0)
    p, f = min(tile_ap.shape[0], BL), min(tile_ap.shape[1], H)
    nc.sync.dma_start(out1[0:p, 0:f], tile_ap[0:p, 0:f])
    nc.sync.dma_start(out2[:], z2[:])


def _build_nc():
    nc = bacc.Bacc()
    eoh = nc.declare_dram_parameter("eoh", [BL, H, S], BF16, isOutput=False)
    eol = nc.declare_dram_parameter("eol", [BL, H, S], BF16, isOutput=False)
    et = nc.declare_dram_parameter("et", [BL, H, S], F32, isOutput=False)
    w2t = nc.declare_dram_parameter("w2t", [128, EC * 2 * H], F32, isOutput=False)
    l2t = nc.declare_dram_parameter("l2t", [128, HC * V], F32, isOutput=False)
    ycst = nc.declare_dram_parameter("ycst", [128, OC], F32, isOutput=False)
    pec2 = nc.declare_dram_parameter("pec2", [128, HC], F32, isOutput=False)
    t1t = nc.declare_dram_parameter("t1t", [128, HC * BL], F32, isOutput=False)
    ident = nc.declare_dram_parameter("ident", [128, 128], F32, isOutput=False)
    onehot = nc.declare_dram_parameter("onehot", [BL, BL * 128], F32, isOutput=False)
    colmask = nc.declare_dram_parameter("colmask", [128, BL * BL], BF16,
                                        isOutput=False)
    l2b = nc.declare_dram_parameter("l2b", [1, V], F32, isOutput=False)
    out1 = nc.declare_dram_parameter("out1", [BL, H], F32, isOutput=True)
    out2 = nc.declare_dram_parameter("out2", [BL, V], F32, isOutput=True)

    with ExitStack() as ctx:
        tc = ctx.enter_context(TileContext(nc))
        const = ctx.enter_context(tc.tile_pool(name="const", bufs=1))
        state = ctx.enter_context(tc.tile_pool(name="state", bufs=1))
        scr = ctx.enter_context(tc.tile_pool(name="scr", bufs=2))
        big = ctx.enter_context(tc.tile_pool(name="big", bufs=1))
        eop = ctx.enter_context(tc.tile_pool(name="eop", bufs=8))
        etp = ctx.enter_context(tc.tile_pool(name="etp", bufs=8))
        psp = ctx.enter_context(tc.tile_pool(name="psp", bufs=2, space="PSUM"))

        w2t_sb = const.tile([128, EC * 2 * H], F32, name="w2t_sb")
        nc.sync.dma_start(w2t_sb[:], w2t[:])
        l2t_sb = const.tile([128, HC * V], F32, name="l2t_sb")
        nc.sync.dma_start(l2t_sb[:], l2t[:])
        ycst_sb = const.tile([128, OC], F32, name="ycst_sb")
        nc.sync.dma_start(ycst_sb[:], ycst[:])
        pec2_sb = const.tile([128, HC], F32, name="pec2_sb")
        nc.sync.dma_start(pec2_sb[:], pec2[:])
        t1t_sb = const.tile([128, HC * BL], F32, name="t1t_sb")
        nc.sync.dma_start(t1t_sb[:], t1t[:])
        ident_sb = const.tile([128, 128], F32, name="ident_sb")
        nc.sync.dma_start(ident_sb[:], ident[:])
        onehot_sb = const.tile([BL, BL * 128], F32, name="onehot_sb")
        nc.sync.dma_start(onehot_sb[:], onehot[:])
        colmask_sb = const.tile([128, BL * BL], BF16, name="colmask_sb")
        nc.sync.dma_start(colmask_sb[:], colmask[:])
        l2b_sb = const.tile([1, V], F32, name="l2b_sb")
        nc.sync.dma_start(l2b_sb[:], l2b[:])
        ones_sb = const.tile([1, 128], F32, name="ones_sb")
        nc.vector.memset(ones_sb[:], 1.0)

        for rep in range(REPS):
            r = f"r{rep}_"
            tT = [t1t_sb[:, hc * BL:(hc + 1) * BL] for hc in range(HC)]
            hT = None
            for l in range(NLAYERS):
                # ---- conv (single live window column) + GLU + residual ----
                y_ps = psp.tile([128, OC * BL], F32, name=f"{r}y_ps{l}", tag="ps")
                for oc in range(OC):
                    for ec in range(EC):
                        nc.tensor.matmul(
                            y_ps[:, oc * BL:(oc + 1) * BL],
                            w2t_sb[:, ec * 2 * H + oc * 128:
                                   ec * 2 * H + (oc + 1) * 128],
                            tT[ec],
                            start=(ec == 0), stop=(ec == EC - 1))
                glu, dT = [], []
                for j in range(HC):
                    sig = scr.tile([128, BL], F32, name="sig", tag="sig")
                    nc.scalar.activation(sig[:],
                                         y_ps[:, (HC + j) * BL:(HC + j + 1) * BL],
                                         AF.Sigmoid,
                                         bias=ycst_sb[:, HC + j:HC + j + 1])
                    av = scr.tile([128, BL], F32, name="av", tag="av")
                    nc.scalar.activation(av[:], y_ps[:, j * BL:(j + 1) * BL],
                                         AF.Identity, bias=ycst_sb[:, j:j + 1])
                    g = state.tile([128, BL], F32, name=f"{r}glu{l}_{j}",
                                   tag=f"{r}glu{l}_{j}")
                    nc.vector.tensor_tensor(g[:], av[:], sig[:], op=OP.mult)
                    d = state.tile([128, BL], F32, name=f"{r}dT{l}_{j}",
                                   tag=f"{r}dT{l}_{j}")
                    nc.vector.tensor_tensor(d[:], g[:], tT[j], op=OP.add)
                    glu.append(g)
                    dT.append(d)

                # hi/lo bf16 split of d (~16-bit effective mantissa)
                dh, dl = [], []
                for j in range(HC):
                    h_ = state.tile([128, BL], BF16, name=f"{r}dh{l}_{j}",
                                    tag=f"{r}dh{l}_{j}")
                    nc.scalar.copy(h_[:], dT[j][:])
                    lo = state.tile([128, BL], BF16, name=f"{r}dl{l}_{j}",
                                    tag=f"{r}dl{l}_{j}")
                    nc.vector.tensor_tensor(lo[:], dT[j][:], h_[:], op=OP.subtract)
                    dh.append(h_)
                    dl.append(lo)

                if STAGE == 0:
                    _dump(nc, scr, out1, out2, dT[0])
                    return nc

                # ---- attention scores (three bf16 cross-term matmuls) ----
                sc_ps = psp.tile([BL, S], F32, name=f"{r}sc_ps{l}", tag="ps")
                for b in range(BL):
                    eoh_ts, eol_ts = [], []
                    eng = nc.sync if b % 2 == 0 else nc.scalar
                    for hc in range(HC):
                        eh_t = eop.tile([128, S], BF16, name="eh_t", tag="eoh")
                        eng.dma_start(eh_t[:],
                                      eoh[b, hc * 128:(hc + 1) * 128, :])
                        el_t = eop.tile([128, S], BF16, name="el_t", tag="eol")
                        eng.dma_start(el_t[:],
                                      eol[b, hc * 128:(hc + 1) * 128, :])
                        eoh_ts.append(eh_t)
                        eol_ts.append(el_t)
                    dmh, dml = [], []
                    for hc in range(HC):
                        mh = scr.tile([128, BL], BF16, name="dmh",
                                      tag=f"dmh_{hc}", bufs=2)
                        nc.vector.tensor_tensor(
                            mh[:], dh[hc][:], colmask_sb[:, b * BL:(b + 1) * BL],
                            op=OP.mult)
                        ml_ = scr.tile([128, BL], BF16, name="dml",
                                       tag=f"dml_{hc}", bufs=2)
                        nc.vector.tensor_tensor(
                            ml_[:], dl[hc][:], colmask_sb[:, b * BL:(b + 1) * BL],
                            op=OP.mult)
                        dmh.append(mh)
                        dml.append(ml_)
                    for hc in range(HC):
                        for wt, rhs_t in ((dmh[hc], eoh_ts[hc]),
                                          (dmh[hc], eol_ts[hc]),
                                          (dml[hc], eoh_ts[hc])):
                            for st in range(ST):
                                sl = slice(st * 512, (st + 1) * 512)
                                start = (b == 0 and hc == 0
                                         and wt is dmh[hc] and rhs_t is eoh_ts[hc])
                                stop = (b == BL - 1 and hc == HC - 1
                                        and wt is dml[hc])
                                nc.tensor.matmul(sc_ps[:, sl], wt[:],
                                                 rhs_t[:, sl],
                                                 start=start, stop=stop)

                # ---- softmax along s (read directly from PSUM) ----
                negmax = scr.tile([BL, 1], F32, name="negmax", tag="negmax")
                nc.vector.tensor_reduce(negmax[:], sc_ps[:], axis=AX.X,
                                        op=OP.max, negate=True)
                expt = big.tile([BL, S], F32, name="expt", tag="expt")
                sumexp = scr.tile([BL, 1], F32, name="sumexp", tag="sumexp")
                nc.scalar.activation(expt[:], sc_ps[:], AF.Exp, bias=negmax[:],
                                     accum_out=sumexp[:])
                rsum = scr.tile([BL, 1], F32, name="rsum", tag="rsum")
                nc.vector.reciprocal(rsum[:], sumexp[:])
                attn = expt
                nc.vector.tensor_scalar_mul(attn[:], expt[:], rsum[:])

                if STAGE == 1:
                    _dump(nc, scr, out1, out2, attn)
                    return nc

                # ---- context: c[b,h] = sum_s attn[b,s] * et[b,h,s] ----
                cT = [state.tile([128, BL], F32, name=f"{r}cT{l}_{hc}",
                                 tag=f"{r}cT{l}_{hc}") for hc in range(HC)]
                for b in range(BL):
                    et_ts = []
                    eng = nc.scalar if b % 2 == 0 else nc.sync
                    for hc in range(HC):
                        et_t = etp.tile([128, S], F32, name="et_t", tag="et")
                        eng.dma_start(et_t[:],
                                      et[b, hc * 128:(hc + 1) * 128, :])
                        et_ts.append(et_t)
                    # broadcast attn row b to all 128 partitions (one-hot mm)
                    bc_ps = psp.tile([128, S], F32, name="bc_ps", tag="ps")
                    for st in range(ST):
                        sl = slice(st * 512, (st + 1) * 512)
                        nc.tensor.matmul(bc_ps[:, sl],
                                         onehot_sb[:, b * 128:(b + 1) * 128],
                                         attn[:, sl], start=True, stop=True)
                    for hc in range(HC):
                        ttr_out = scr.tile([128, S], F32, name="ttr_out",
                                           tag="ttr_out", bufs=1)
                        nc.vector.affine_mul_reduce(
                            out=ttr_out[:],
                            accum_out=cT[hc][:, b:b + 1],
                            in0=et_ts[hc][:],
                            in1=bc_ps[:],
                            scale=1.0, bias=0.0)

                if STAGE == 2:
                    _dump(nc, scr, out1, out2, cT[0])
                    return nc

                # ---- h = glu + c ; next-layer t = h + pe3[:,2] ----
                hT = []
                for hc in range(HC):
                    h = state.tile([128, BL], F32, name=f"{r}hT{l}_{hc}",
                                   tag=f"{r}hT{l}_{hc}")
                    nc.vector.tensor_tensor(h[:], glu[hc][:], cT[hc][:],
                                            op=OP.add)
                    hT.append(h)
                if l < NLAYERS - 1:
                    tT = []
                    for hc in range(HC):
                        t = state.tile([128, BL], F32, name=f"{r}tT{l + 1}_{hc}",
                                       tag=f"{r}tT{l + 1}_{hc}")
                        nc.vector.tensor_scalar_add(t[:], hT[hc][:],
                                                    pec2_sb[:, hc:hc + 1])
                        tT.append(t)

            # ---- out1 = softmax(h3, axis=h) ----
            eh = []
            for hc in range(HC):
                e_t = state.tile([128, BL], F32, name=f"{r}eh_{hc}",
                                 tag=f"{r}eh_{hc}")
                nc.scalar.activation(e_t[:], hT[hc][:], AF.Exp)
                eh.append(e_t)
            tp_ps = psp.tile([BL, H], F32, name=f"{r}tp_ps", tag="ps")
            for hc in range(HC):
                nc.tensor.matmul(tp_ps[:, hc * 128:(hc + 1) * 128], eh[hc][:],
                                 ident_sb[:], is_transpose=True,
                                 start=True, stop=True)
            se = scr.tile([BL, 1], F32, name="se", tag="se")
            nc.vector.reduce_sum(se[:], tp_ps[:], axis=AX.X)
            rse = scr.tile([BL, 1], F32, name="rse", tag="rse")
            nc.vector.reciprocal(rse[:], se[:])
            out1_sb = scr.tile([BL, H], F32, name="out1_sb", tag="out1_sb")
            nc.vector.tensor_scalar_mul(out1_sb[:], tp_ps[:], rse[:])
            nc.sync.dma_start(out1[:], out1_sb[:])

            # ---- out2 = log_softmax(h3 @ lin2_w.T + lin2_b) ----
            lg_ps = psp.tile([BL, V], F32, name=f"{r}lg_ps", tag="ps")
            for hc in range(HC):
                nc.tensor.matmul(lg_ps[:], hT[hc][:],
                                 l2t_sb[:, hc * V:(hc + 1) * V],
                                 start=(hc == 0), stop=False)
            nc.tensor.matmul(lg_ps[:], ones_sb[:, :BL], l2b_sb[:],
                             start=False, stop=True)
            negml = scr.tile([BL, 1], F32, name="negml", tag="negml")
            nc.vector.tensor_reduce(negml[:], lg_ps[:], axis=AX.X, op=OP.max,
                                    negate=True)
            exl = scr.tile([BL, V], F32, name="exl", tag="exl")
            sel = scr.tile([BL, 1], F32, name="sel", tag="sel")
            nc.scalar.activation(exl[:], lg_ps[:], AF.Exp, bias=negml[:],
                                 accum_out=sel[:])
            lsel = scr.tile([BL, 1], F32, name="lsel", tag="lsel")
            nc.scalar.activation(lsel[:], sel[:], AF.Ln)
            out2_sb = scr.tile([BL, V], F32, name="out2_sb", tag="out2_sb")
            nc.vector.tensor_scalar(out2_sb[:], lg_ps[:], negml[:], lsel[:],
                                    op0=OP.add, op1=OP.subtract)
            nc.sync.dma_start(out2[:], out2_sb[:])

    return nc


def _host_prep(inputs):
    pe3 = _pos_emb(K, E)                                    # [512, 3]
    emb = np.asarray(inputs["emb"], np.float32)
    conv_w = np.asarray(inputs["conv_w"], np.float32)
    conv_b = np.asarray(inputs["conv_b"], np.float32)
    l2w = np.asarray(inputs["lin2_w"], np.float32)
    l2bv = np.asarray(inputs["lin2_b"], np.float32)
    tok = np.asarray(inputs["decoder_input"]).astype(np.int64)

    c0 = emb[PAD] + pe3[:, 0]
    c1 = emb[PAD] + pe3[:, 1]
    y_const = conv_w[:, :, 0] @ c0 + conv_w[:, :, 1] @ c1 + conv_b   # [1024]
    w2 = conv_w[:, :, 2]                                    # [1024, 512]
    w2t = np.ascontiguousarray(
        np.transpose(w2.reshape(2 * H, EC, 128), (2, 1, 0)).reshape(128, EC * 2 * H))
    l2t = np.ascontiguousarray(
        np.transpose(l2w.reshape(V, HC, 128), (2, 1, 0)).reshape(128, HC * V))
    ycst = np.ascontiguousarray(y_const.reshape(OC, 128).T)
    pec2 = np.ascontiguousarray(pe3[:, 2].reshape(HC, 128).T)
    t1 = emb[tok] + pe3[:, 2][None, :]                      # [B, 512]
    ident = np.eye(128, dtype=np.float32)
    onehot = np.zeros((BL, BL * 128), dtype=np.float32)
    colmask = np.zeros((128, BL * BL), dtype=ml_dtypes.bfloat16)
    for b in range(BL):
        onehot[b, b * 128:(b + 1) * 128] = 1.0
        colmask[:, b * BL + b] = 1.0
    l2b = np.ascontiguousarray(l2bv.reshape(1, V))
    return w2t, l2t, ycst, pec2, t1, ident, onehot, colmask, l2b


def _make_in_maps(inputs):
    eo_full = np.asarray(inputs["encoder_output"], np.float32)
    eoh_full = eo_full.astype(ml_dtypes.bfloat16)
    eol_full = (eo_full - eoh_full.astype(np.float32)).astype(ml_dtypes.bfloat16)
    et_full = np.ascontiguousarray(np.asarray(inputs["encoder_total"], np.float32))
    w2t, l2t, ycst, pec2, t1, ident, onehot, colmask, l2b = _host_prep(inputs)
    in_maps = []
    for c in range(NCORES):
        sl = slice(c * BL, (c + 1) * BL)
        t1c = t1[sl]                                        # [16, 512]
        t1t = np.ascontiguousarray(
            np.transpose(t1c.reshape(BL, HC, 128), (2, 1, 0)).reshape(128, HC * BL))
        in_maps.append({
            "eoh": np.ascontiguousarray(eoh_full[sl]),
            "eol": np.ascontiguousarray(eol_full[sl]),
            "et": np.ascontiguousarray(et_full[sl]),
            "w2t": w2t, "l2t": l2t, "ycst": ycst, "pec2": pec2,
            "t1t": t1t, "ident": ident, "onehot": onehot, "colmask": colmask,
            "l2b": l2b,
        })
    return in_maps


_NC = None
LAST_RESULTS = None


def kernel(**inputs):
    global _NC, LAST_RESULTS
    import os
    in_maps = _make_in_maps(inputs)

    if _NC is None:
        _NC = _build_nc()
        _NC.finalize()   # Bacc.compile(): wait splitting, event sems, regalloc

    trace = bool(int(os.environ.get("KERNEL_TRACE", "0")))
    res = run_bass_kernel_spmd(_NC, in_maps, list(range(NCORES)), trace=trace)
    LAST_RESULTS = res
    out1 = np.concatenate([np.asarray(r["out1"]) for r in res.results], axis=0)
    out2 = np.concatenate([np.asarray(r["out2"]) for r in res.results], axis=0)
    return (np.ascontiguousarray(out1[:, :, None].astype(np.float32)),
            np.ascontiguousarray(out2[:, None, :].astype(np.float32)))
